# revision 14
# baseline (speedup 1.0000x reference)
"""Trainium2 Bass kernel for AtomActionPredictor: gather + 2-layer MLP.

Strategy (data parallel over 8 NeuronCores):
  - atom_features (cast to bf16) and the tiny MLP weights are replicated;
    each core does a local gather + MLP with no communication.
  - The gather uses the GPSIMD dma_gather custom instruction in transpose
    mode, which lands gathered rows *feature-major* in SBUF ([d, token]) —
    exactly the matmul rhs orientation — so no on-chip transposes.
  - dma_gather indices are int16, so the atom table is processed in banks of
    32768 rows. All indices are host-sorted by bank and dealt round-robin to
    the 8 cores so per-(core, bank) counts are balanced; each bank is padded
    (with index 0) to a common size across cores so all 8 cores run one SPMD
    graph. Gathers round-robin over 4 SWDGE queues.
  - MLP runs in bf16 with f32 PSUM accumulation; GELU (erf) + biases on the
    scalar (ACT) engine.
  - Per-core output layout is [VOCAB, Tpad] in DRAM; the host transposes and
    un-sorts back to token order.
"""
import numpy as np
import ml_dtypes

import concourse.bass as bass
import concourse.mybir as mybir
import concourse.tile as tile
from concourse import bacc
from concourse.bass_utils import run_bass_kernel_spmd

N_CORES = 8
NUM_ATOMS = 1_000_000
ATOM_DIM = 256
HIDDEN = 128
VOCAB = 128
TOTAL_RC = 400_000

BANK_ROWS = 32768          # int16-addressable bank height
G_SUB = 896                # max indices per dma_gather (transpose mode)
N_CHUNK = 512              # matmul moving-dim chunk (one PSUM bank)
N_QUEUES = 4               # SWDGE queues for gather round-robin

F32 = mybir.dt.float32
BF16 = mybir.dt.bfloat16
I16 = mybir.dt.int16


def _round_up(x, m):
    return (x + m - 1) // m * m


def build_graph(num_atoms, bank_sizes, out_dtype=BF16, act_fn=None):
    """Build the SPMD graph. bank_sizes[b] = padded token count for bank b
    (each a positive multiple of 128, identical across cores)."""
    if act_fn is None:
        act_fn = mybir.ActivationFunctionType.Gelu
    n_banks = len(bank_sizes)
    tpad = int(sum(bank_sizes))
    assert num_atoms <= n_banks * BANK_ROWS

    nc = bacc.Bacc("TRN2", target_bir_lowering=False, debug=False,
                   num_devices=N_CORES, num_swdge_queues=N_QUEUES,
                   dynamic_dma_scratch_size=49152)
    feats = nc.dram_tensor("feats", [num_atoms, ATOM_DIM], BF16,
                           kind="ExternalInput")
    idx16 = nc.dram_tensor("idx16", [128, tpad // 16], I16,
                           kind="ExternalInput")
    w1p = nc.dram_tensor("w1p", [128, 2 * HIDDEN], BF16, kind="ExternalInput")
    w2p = nc.dram_tensor("w2p", [HIDDEN, VOCAB], BF16, kind="ExternalInput")
    b1p = nc.dram_tensor("b1p", [HIDDEN, 1], F32, kind="ExternalInput")
    b2p = nc.dram_tensor("b2p", [VOCAB, 1], F32, kind="ExternalInput")
    out = nc.dram_tensor("out", [VOCAB, tpad], out_dtype,
                         kind="ExternalOutput")

    with tile.TileContext(nc) as tc:
        with (
            tc.tile_pool(name="const", bufs=1) as cpool,
            tc.tile_pool(name="xt", bufs=8) as xt_pool,
            tc.tile_pool(name="ht", bufs=6) as ht_pool,
            tc.tile_pool(name="osb", bufs=6) as osb_pool,
            tc.tile_pool(name="psh", bufs=2, space="PSUM") as psh_pool,
            tc.tile_pool(name="pso", bufs=2, space="PSUM") as pso_pool,
        ):
            w1_sb = cpool.tile([128, 2 * HIDDEN], BF16)
            nc.sync.dma_start(out=w1_sb[:], in_=w1p[:])
            w2_sb = cpool.tile([HIDDEN, VOCAB], BF16)
            nc.sync.dma_start(out=w2_sb[:], in_=w2p[:])
            b1_sb = cpool.tile([HIDDEN, 1], F32)
            nc.sync.dma_start(out=b1_sb[:], in_=b1p[:])
            b2_sb = cpool.tile([VOCAB, 1], F32)
            nc.sync.dma_start(out=b2_sb[:], in_=b2p[:])
            idx_sb = cpool.tile([128, tpad // 16], I16)
            nc.sync.dma_start(out=idx_sb[:], in_=idx16[:])

            off = 0
            g_i = 0
            for b in range(n_banks):
                pb = int(bank_sizes[b])
                row0 = b * BANK_ROWS
                rows = min(BANK_ROWS, num_atoms - row0)
                n_sub = -(-pb // G_SUB)
                base = pb // n_sub // 128 * 128
                sizes = [base] * n_sub
                for k in range((pb - base * n_sub) // 128):
                    sizes[k] += 128
                for g in sizes:
                    # xt[p, c, t] = feats[row0 + idx[t]][c*128 + p]  (bf16)
                    xt = xt_pool.tile([128, 2, g], BF16, tag="xt")
                    nc.gpsimd.dma_gather(
                        out_ap=xt[:],
                        in_ap=feats[row0:row0 + rows, :],
                        idxs_ap=idx_sb[:, off // 16:(off + g) // 16],
                        num_idxs=g,
                        num_idxs_reg=g,
                        elem_size=ATOM_DIM,
                        transpose=True,
                        queue_num=g_i % N_QUEUES,
                    )
                    g_i += 1
                    ht = ht_pool.tile([HIDDEN, g], BF16, tag="ht")
                    osb = osb_pool.tile([VOCAB, g], out_dtype, tag="osb")
                    for n0 in range(0, g, N_CHUNK):
                        n = min(N_CHUNK, g - n0)
                        ps_h = psh_pool.tile([HIDDEN, n], F32, tag="psh")
                        nc.tensor.matmul(ps_h[:], lhsT=w1_sb[:, 0:HIDDEN],
                                         rhs=xt[:, 0, n0:n0 + n],
                                         start=True, stop=False)
                        nc.tensor.matmul(ps_h[:], lhsT=w1_sb[:, HIDDEN:2 * HIDDEN],
                                         rhs=xt[:, 1, n0:n0 + n],
                                         start=False, stop=True)
                        nc.scalar.activation(ht[:, n0:n0 + n], ps_h[:],
                                             act_fn,
                                             bias=b1_sb[:, 0:1], scale=1.0)
                        ps_o = pso_pool.tile([VOCAB, n], F32, tag="pso")
                        nc.tensor.matmul(ps_o[:], lhsT=w2_sb[:],
                                         rhs=ht[:, n0:n0 + n],
                                         start=True, stop=True)
                        nc.vector.tensor_tensor(
                            out=osb[:, n0:n0 + n], in0=ps_o[:],
                            in1=b2_sb[:, 0:1].to_broadcast([VOCAB, n]),
                            op=mybir.AluOpType.add)
                    nc.sync.dma_start(out=out[:, off:off + g], in_=osb[:])
                    off += g
    nc.compile()
    return nc


def _prep_indices(rc_indices, num_atoms):
    """Globally sort indices by bank, deal each bank's tokens round-robin
    across cores (balances per-core bank counts), pad each bank (with index
    0) to a common per-core size, and build the wrapped int16 index params.

    Returns (idx16_per_core, token_map, bank_sizes, tpad).
    token_map[c] maps padded slot -> global token id (-1 for pad slots).
    """
    n_banks = (num_atoms + BANK_ROWS - 1) // BANK_ROWS

    banks = rc_indices >> 15
    order = np.argsort(banks, kind="stable")        # global tokens by bank
    counts = np.bincount(banks, minlength=n_banks)  # per-bank totals

    bank_sizes = []
    for b in range(n_banks):
        per_core_max = -(-int(counts[b]) // N_CORES)  # ceil
        bank_sizes.append(max(128, _round_up(per_core_max, 128)))
    tpad = int(sum(bank_sizes))

    lin = np.zeros((N_CORES, tpad), np.int16)
    token_map = np.full((N_CORES, tpad), -1, np.int64)
    pos = 0
    off = 0
    for b in range(n_banks):
        cnt = int(counts[b])
        toks = order[pos:pos + cnt]                  # global token ids, bank b
        within = (rc_indices[toks] & (BANK_ROWS - 1)).astype(np.int16)
        for c in range(N_CORES):
            sl = slice(c, cnt, N_CORES)              # round-robin deal
            k = len(range(*sl.indices(cnt)))
            lin[c, off:off + k] = within[sl]
            token_map[c, off:off + k] = toks[sl]
        pos += cnt
        off += bank_sizes[b]

    idx16_per_core = []
    for c in range(N_CORES):
        wrapped = lin[c].reshape(tpad // 16, 16).T   # [16, tpad//16]
        idx16_per_core.append(np.tile(wrapped, (8, 1)).copy())
    return idx16_per_core, token_map, bank_sizes, tpad


def kernel(atom_features, rc_indices, W1, b1, W2, b2):
    num_atoms = atom_features.shape[0]
    rc_indices = np.asarray(rc_indices)
    n_rc = rc_indices.shape[0]

    idx16s, token_map, bank_sizes, tpad = _prep_indices(rc_indices, num_atoms)

    nc = build_graph(num_atoms, bank_sizes)

    feats_bf = np.asarray(atom_features).astype(ml_dtypes.bfloat16)
    w1p = np.ascontiguousarray(
        np.asarray(W1).reshape(2, 128, HIDDEN).transpose(1, 0, 2)
        .reshape(128, 2 * HIDDEN)).astype(ml_dtypes.bfloat16)
    w2p = np.asarray(W2).astype(ml_dtypes.bfloat16)
    b1p = np.asarray(b1).reshape(HIDDEN, 1).astype(np.float32)
    b2p = np.asarray(b2).reshape(VOCAB, 1).astype(np.float32)

    in_maps = [{"feats": feats_bf, "idx16": idx16s[c], "w1p": w1p,
                "w2p": w2p, "b1p": b1p, "b2p": b2p} for c in range(N_CORES)]
    res = run_bass_kernel_spmd(nc, in_maps, core_ids=list(range(N_CORES)))

    logits = np.empty((n_rc, VOCAB), np.float32)
    for c in range(N_CORES):
        oc = res.results[c]["out"]  # [VOCAB, tpad]
        valid = token_map[c] >= 0
        logits[token_map[c][valid]] = oc[:, valid].T.astype(np.float32)
    return logits
